# revision 10
# baseline (speedup 1.0000x reference)
"""Trainium2 Bass kernel for nn_AttentionLayer (B=4, S=4096, D=128, fp32).

Sharding: batch (4) x query-half (2) across 8 NeuronCores; the query half is
realized by a host-side column ROTATION of x^T (keys are permutation
invariant under softmax+sum), so every core runs the identical SPMD program
with its queries at columns 0..sq-1.

Math restructure vs a direct port:
  scores[t,s] = q_s . k_t = x_s^T (Wq^T Wk) x_t
    -> precompute (host, fp64) gT = Wk^T Wq; on device GX = gT^T @ x^T once
       (a [128,4096] tensor), then scores chunks = GX-chunk^T @ x^T.
       This removes the Q and K projections, their PSUM->SBUF copies, and
       the duplicated query DMA of the v1 kernel.
  bq is folded into the exp bias alpha[t] = SCALE*bq.k_t (extra column of
    the V projection, as v1); bk cancels in softmax; bv is applied on host.
  exp is computed straight into fp8e4 (range centered by a global shift C,
    which cancels exactly in softmax); the AV and denominator matmuls then
    run in fp8 DoubleRow perf mode (2 contraction rows/partition, 0.5
    cycles/col). V is split into an fp8 hi+lo pair (error feedback), so V
    quantization contributes ~nothing; only the exp fp8 rounding remains.
  Normalization (num/den + bv) happens on host in fp64 for free.

Engine budget per core: PE ~123k cycles (scores 64k, AV+den 49k, proj 8k);
ACT all-exp would be ~80k cycles, so a slice of exp chunks is offloaded as
int16-Schraudolph (DVE/Pool tensor_scalar -> bf16 bit pattern) + bf16->fp8
convert on the other of the two engines.
"""

import sys

import numpy as np

for _p in ("/opt/trn_rl_repo", "/opt/pypackages"):
    if _p not in sys.path:
        sys.path.append(_p)

B, S, D = 4, 4096, 128
N_CORES = 8
SQ = S // 2            # queries per core
SCALE = 1.0 / float(np.sqrt(D))
CSHIFT = 1.5           # global exp shift: exp(y-C); cancels in softmax
# Schraudolph (bf16 bit pattern): i16 = y*184.6635 + 16256.5 + delta
SCH_A = 128.0 / float(np.log(2.0))
SCH_DELTA = -5.5       # centers the 2^frac linear-interp overestimate
LDW = True


def default_exp_sched(n_pass, tch, n_dve=0):
    """Per (pass, chunk) exp engine: 'act' or 'dve' (Schraudolph+convert).
    The first chunks of each pass stay on ACT (critical path); offloaded
    chunks are spread through the middle/end of the pass."""
    sched = {}
    for p in range(n_pass):
        dve_set = set()
        if n_dve > 0:
            # spread n_dve chunks over chunk indices 2..tch-1
            step = max(1, (tch - 2) // n_dve)
            c = 2
            while len(dve_set) < n_dve and c < tch:
                dve_set.add(c)
                c += step
        for c in range(tch):
            sched[(p, c)] = "dve" if c in dve_set else "act"
    return sched


def build_attention_bass(s=S, sq=SQ, sw=1024, n_dve_exp=8):
    """Single-core SPMD program. s: keys; sq: queries; sw: pass width."""
    import concourse.bass as bass
    import concourse.mybir as mybir
    import concourse.tile as tile
    from concourse import bacc
    from contextlib import ExitStack

    f32 = mybir.dt.float32
    f32r = mybir.dt.float32r
    fp8 = mybir.dt.float8e4
    bf16 = mybir.dt.bfloat16
    i16 = mybir.dt.int16
    FT = mybir.ActivationFunctionType
    DR = mybir.MatmulPerfMode.DoubleRow
    ALU = mybir.AluOpType

    tch = s // 128          # key chunks (128 keys each)
    npair = tch // 2        # 256-key pair groups
    n_pass = sq // sw
    nw = min(512, sw)       # matmul N width (f32r needs >=256)
    jn = sw // nw
    gxw = min(512, s)       # GX matmul chunk width
    sched = default_exp_sched(n_pass, tch, n_dve_exp)

    nc = bacc.Bacc("TRN2", target_bir_lowering=False, debug=False)

    xT = nc.dram_tensor("xT", [D, s], f32r, kind="ExternalInput").ap()
    gT = nc.dram_tensor("gT", [D, D], f32r, kind="ExternalInput").ap()
    wvT = nc.dram_tensor("wvT", [D, D + 2], f32r, kind="ExternalInput").ap()
    num_d = nc.dram_tensor("num", [D, sq], f32, kind="ExternalOutput").ap()
    den_d = nc.dram_tensor("den", [1, sq], f32, kind="ExternalOutput").ap()

    with tile.TileContext(nc) as tc, ExitStack() as ctx:
        const = ctx.enter_context(tc.tile_pool(name="const", bufs=1))
        big = ctx.enter_context(tc.tile_pool(name="big", bufs=1))
        exp_pool = ctx.enter_context(tc.tile_pool(name="exp", bufs=4))
        sch_pool = ctx.enter_context(tc.tile_pool(name="sch", bufs=3))
        vres_pool = ctx.enter_context(tc.tile_pool(name="vres", bufs=2))
        stage = ctx.enter_context(tc.tile_pool(name="stage", bufs=2))

        gT_sb = const.tile([D, D], f32r, tag="gT")
        wv_sb = const.tile([D, D + 2], f32r, tag="wv")
        ones8 = const.tile([128, 256], fp8, tag="ones8")
        alpha_sb = const.tile([128, tch], f32, tag="alpha")    # alpha - C
        alpha16 = const.tile([128, tch], f32, tag="alpha16")   # schraudolph bias

        xT_sb = big.tile([D, s], f32r, tag="xT")
        gx_sb = big.tile([D, s], f32r, tag="gx")
        v8hi = big.tile([128, s], fp8, tag="v8hi")   # [(pair g, two, e128)]
        v8lo = big.tile([128, s], fp8, tag="v8lo")

        # ---- input DMAs: few large transfers (SWDGE descriptor generation
        # is ~1us per dma_start and serializes on the queue engine)
        nc.sync.dma_start(gT_sb[:], gT)
        nc.sync.dma_start(wv_sb[:], wvT)
        xw = max(1024, s // 4)
        for st in range(0, s, xw):
            w = min(xw, s - st)
            nc.sync.dma_start(xT_sb[:, st:st + w], xT[:, st:st + w])
        nc.vector.memset(ones8[:], 1.0)

        # ---- phase A: GX projection + V/alpha, PSUM pools closed after
        qkv_ctx = ExitStack()
        gxps = qkv_ctx.enter_context(tc.tile_pool(name="gxps", bufs=3,
                                                  space="PSUM"))
        vps = qkv_ctx.enter_context(tc.tile_pool(name="vps", bufs=3,
                                                 space="PSUM"))

        def emit_gx(j):
            st, w = j * gxw, min(gxw, s - j * gxw)
            gp = gxps.tile([128, gxw], f32, tag="gx")
            nc.tensor.matmul(gp[:, :w], gT_sb[:], xT_sb[:, st:st + w])
            nc.vector.tensor_copy(gx_sb[:, st:st + w], gp[:, :w])

        def emit_v(c):
            vp = vps.tile([128, D + 2], f32, tag="vp")
            xc = xT_sb[:, c * 128:(c + 1) * 128]
            nc.tensor.matmul(vp[:], xc, wv_sb[:])
            # GPSIMD cannot touch PSUM (walrus birverifier) -> all on DVE
            nc.vector.tensor_copy(v8hi[:, c * 128:(c + 1) * 128], vp[:, :D])
            vr = vres_pool.tile([128, D], f32, tag="vr")
            nc.vector.tensor_sub(vr[:], vp[:, :D],
                                 v8hi[:, c * 128:(c + 1) * 128])
            nc.vector.tensor_copy(v8lo[:, c * 128:(c + 1) * 128], vr[:])
            nc.vector.tensor_scalar_add(alpha_sb[:, c:c + 1],
                                        vp[:, D:D + 1], -CSHIFT)

        ngx = (s + gxw - 1) // gxw
        # GX chunk 0 + V chunks 0/1 feed the first scores/AV pair
        emit_gx(0)
        emit_v(0)
        emit_v(1)
        for j in range(1, ngx):
            emit_gx(j)
        for c in range(2, tch):
            emit_v(c)
        qkv_ctx.close()

        # schraudolph per-partition bias from alpha (single DVE op)
        nc.vector.tensor_scalar(alpha16[:], alpha_sb[:], SCH_A,
                                16256.5 + SCH_DELTA, ALU.mult, ALU.add)

        acc_ctx = ExitStack()
        scps = acc_ctx.enter_context(tc.tile_pool(name="scps", bufs=2,
                                                  space="PSUM"))
        accps = acc_ctx.enter_context(tc.tile_pool(name="accps", bufs=1,
                                                   space="PSUM"))
        denps = acc_ctx.enter_context(tc.tile_pool(name="denps", bufs=1,
                                                   space="PSUM"))

        ones3 = ones8[:, :].rearrange("p (two e) -> p two e", two=2)

        def emit_scores(p, c):
            sc = scps.tile([128, sw], f32, tag="sc")
            gxc = gx_sb[:, c * 128:(c + 1) * 128]
            for j in range(jn):
                nc.tensor.matmul(sc[:, j * nw:(j + 1) * nw], gxc,
                                 xT_sb[:, p * sw + j * nw:
                                       p * sw + (j + 1) * nw])
            return sc

        def emit_exp(p, c, sc, pair):
            """exp(SCALE*sc + alpha[c] - C) -> fp8 into pair slot c%2."""
            half = pair[:, (c % 2) * sw:(c % 2) * sw + sw]
            if sched[(p, c)] == "act":
                nc.scalar.activation(half, sc[:], FT.Exp,
                                     bias=alpha_sb[:, c:c + 1], scale=SCALE)
            else:
                # DVE schraudolph -> bf16 bits; Pool converts bf16 -> fp8
                t16 = sch_pool.tile([128, sw], i16, tag="t16")
                nc.vector.tensor_scalar(t16[:], sc[:], SCALE * SCH_A,
                                        alpha16[:, c:c + 1],
                                        ALU.mult, ALU.add)
                nc.gpsimd.tensor_copy(half, t16[:].bitcast(bf16))

        # ---- attention passes
        pairs = {}
        for p in range(n_pass):
            acc_o = accps.tile([128, sw], f32, tag="acco")
            den_ps = denps.tile([128, sw], f32, tag="den")

            def emit_av(p, g, first, last):
                # same-stationary matmuls kept adjacent (ldweights reuse)
                pair3 = pairs.pop((p, g))[:, :].rearrange(
                    "p (two n) -> p two n", two=2)
                vh = v8hi[:, g * 256:(g + 1) * 256].rearrange(
                    "p (two e) -> p two e", two=2)
                vl = v8lo[:, g * 256:(g + 1) * 256].rearrange(
                    "p (two e) -> p two e", two=2)
                for st, kw in ((vh, dict(start=first, stop=False)),
                               (vl, dict(start=False, stop=last))):
                    for j in range(jn):
                        nc.tensor.matmul(acc_o[:, j * nw:(j + 1) * nw], st,
                                         pair3[:, :, j * nw:(j + 1) * nw],
                                         perf_mode=DR, **kw)
                for j in range(jn):
                    nc.tensor.matmul(den_ps[:, j * nw:(j + 1) * nw],
                                     ones3, pair3[:, :, j * nw:(j + 1) * nw],
                                     perf_mode=DR, start=first, stop=last)

            for c in range(tch):
                sc = emit_scores(p, c)
                g = c // 2
                if c % 2 == 0:
                    pairs[(p, g)] = exp_pool.tile([128, 2 * sw], fp8,
                                                  name="pair", tag="pair")
                emit_exp(p, c, sc, pairs[(p, g)])
                if c % 2 == 1:
                    emit_av(p, g, first=(g == 0), last=(g == npair - 1))

            # stage num/den to SBUF (Pool/DVE), DMA out
            num_sb = stage.tile([128, sw], f32, tag="num")
            den_sb = stage.tile([1, sw], f32, tag="densb")
            nc.vector.tensor_copy(num_sb[:], acc_o[:])
            nc.vector.tensor_copy(den_sb[:], den_ps[0:1, :])
            nc.sync.dma_start(num_d[:, p * sw:(p + 1) * sw], num_sb[:])
            nc.sync.dma_start(den_d[:, p * sw:(p + 1) * sw], den_sb[:])
        acc_ctx.close()
    nc.compile()
    return nc


def make_in_maps(x, Wq, bq, Wk, Wv, s=S, sq=SQ, n_cores=N_CORES):
    """Per-core inputs. Core c -> batch c//per_b, query half c%per_b via
    column rotation of x^T."""
    x = np.asarray(x, np.float64)
    nb = x.shape[0]
    per_b = n_cores // nb
    Wq = np.asarray(Wq, np.float64)
    Wk = np.asarray(Wk, np.float64)
    Wv = np.asarray(Wv, np.float64)
    bq = np.asarray(bq, np.float64)
    gT = (Wk.T @ Wq).astype(np.float32)               # [d', d]
    wtl = (SCALE * (Wk.T @ bq)).reshape(D, 1)
    wv_aug = np.concatenate([Wv.T, wtl, wtl], axis=1).astype(np.float32)
    maps = []
    for c in range(n_cores):
        b, h = c // per_b, c % per_b
        xt = np.ascontiguousarray(x[b].T.astype(np.float32))
        if h:
            xt = np.ascontiguousarray(
                np.concatenate([xt[:, h * sq:], xt[:, :h * sq]], axis=1))
        maps.append({"xT": xt, "gT": gT, "wvT": wv_aug})
    return maps


_NC_CACHE = {}


def _get_nc():
    if "nc" not in _NC_CACHE:
        _NC_CACHE["nc"] = build_attention_bass()
    return _NC_CACHE["nc"]


def postprocess(results, bv, x_shape=(B, S, D), n_cores=N_CORES, sq=SQ):
    """results[c] = {num: [D, sq], den: [1, sq]} -> full [B, S*D] output."""
    nb = x_shape[0]
    per_b = n_cores // nb
    bv = np.asarray(bv, np.float64).reshape(1, D)
    out = np.empty((nb, x_shape[1] * D), np.float32)
    for c in range(n_cores):
        b, h = c // per_b, c % per_b
        num = np.asarray(results[c]["num"], np.float64)   # [D, sq]
        den = np.asarray(results[c]["den"], np.float64)   # [1, sq]
        o = (num / den).T + bv                            # [sq, D]
        out[b, h * sq * D:(h + 1) * sq * D] = o.astype(np.float32).reshape(-1)
    return out


def run_on_hw(inputs, trace=False, **kw):
    from concourse.bass_utils import run_bass_kernel_spmd
    nc = _get_nc()
    maps = make_in_maps(inputs["x"], inputs["Wq"], inputs["bq"],
                        inputs["Wk"], inputs["Wv"])
    res = run_bass_kernel_spmd(nc, maps, core_ids=list(range(N_CORES)),
                               trace=trace, **kw)
    out = postprocess(res.results, inputs["bv"],
                      x_shape=np.asarray(inputs["x"]).shape)
    return out, res


def kernel(**inputs):
    out, _ = run_on_hw(inputs, trace=False)
    return out


# revision 15
# speedup vs baseline: 1.3333x; 1.3333x over previous
"""Trainium2 Bass kernel for nn_AttentionLayer (B=4, S=4096, D=128, fp32).

Sharding: batch (4) x query-half (2) across 8 NeuronCores; the query half is
realized by a host-side column ROTATION of x^T (keys are permutation
invariant under softmax+sum), so every core runs the identical SPMD program
with its queries at columns 0..sq-1.

Math restructure vs a direct port:
  scores[t,s] = q_s . k_t = x_s^T (Wq^T Wk) x_t
    -> precompute (host, fp64) gT = Wk^T Wq; on device GX = gT^T @ x^T once
       (a [128,4096] tensor), then scores chunks = GX-chunk^T @ x^T.
       This removes the Q and K projections, their PSUM->SBUF copies, and
       the duplicated query DMA of the v1 kernel.
  bq is folded into the exp bias alpha[t] = SCALE*bq.k_t (extra column of
    the V projection, as v1); bk cancels in softmax; bv is applied on host.
  exp is computed straight into fp8e4 (range centered by a global shift C,
    which cancels exactly in softmax); the AV and denominator matmuls then
    run in fp8 DoubleRow perf mode (2 contraction rows/partition, 0.5
    cycles/col). V is split into an fp8 hi+lo pair (error feedback), so V
    quantization contributes ~nothing; only the exp fp8 rounding remains.
  Normalization (num/den + bv) happens on host in fp64 for free.

Engine budget per core: PE ~123k cycles (scores 64k, AV+den 49k, proj 8k);
ACT all-exp would be ~80k cycles, so a slice of exp chunks is offloaded as
int16-Schraudolph (DVE/Pool tensor_scalar -> bf16 bit pattern) + bf16->fp8
convert on the other of the two engines.
"""

import sys

import numpy as np

for _p in ("/opt/trn_rl_repo", "/opt/pypackages"):
    if _p not in sys.path:
        sys.path.append(_p)

B, S, D = 4, 4096, 128
N_CORES = 8
SQ = S // 2            # queries per core
SCALE = 1.0 / float(np.sqrt(D))
CSHIFT = 1.5           # global exp shift: exp(y-C); cancels in softmax
# Schraudolph (bf16 bit pattern): i16 = y*184.6635 + 16256.5 + delta
SCH_A = 128.0 / float(np.log(2.0))
SCH_DELTA = -5.5       # centers the 2^frac linear-interp overestimate
LDW = True


def default_exp_sched(n_pass, tch, n_dve=0):
    """Per (pass, chunk) exp engine: 'act' or 'dve' (Schraudolph + DMA-cast).
    n_dve: int (same per pass) or per-pass list. Pass 0's DVE is busy with
    V-prep, so its offloaded chunks sit late in the pass."""
    if isinstance(n_dve, int):
        n_dve = [n_dve] * n_pass
    sched = {}
    for p in range(n_pass):
        nd = min(n_dve[p] if p < len(n_dve) else n_dve[-1], tch - 2)
        dve_set = set()
        if nd > 0:
            if p == 0:
                # late chunks only (V-prep occupies DVE early in pass 0),
                # but keep the final two pairs on ACT to protect the tail
                hi = max(2, tch - 3)
                dve_set = set(range(hi, max(1, hi - nd), -1))
            else:
                step = max(1, (tch - 2) // nd)
                c = 2
                while len(dve_set) < nd and c < tch:
                    dve_set.add(c)
                    c += step
        for c in range(tch):
            sched[(p, c)] = "dve" if c in dve_set else "act"
    return sched


def build_attention_bass(s=S, sq=SQ, sw=1024, n_dve_exp=(5, 11)):
    """Single-core SPMD program. s: keys; sq: queries; sw: pass width."""
    import concourse.bass as bass
    import concourse.mybir as mybir
    import concourse.tile as tile
    from concourse import bacc
    from contextlib import ExitStack

    f32 = mybir.dt.float32
    f32r = mybir.dt.float32r
    fp8 = mybir.dt.float8e4
    bf16 = mybir.dt.bfloat16
    i16 = mybir.dt.int16
    FT = mybir.ActivationFunctionType
    DR = mybir.MatmulPerfMode.DoubleRow
    ALU = mybir.AluOpType

    tch = s // 128          # key chunks (128 keys each)
    npair = tch // 2        # 256-key pair groups
    n_pass = sq // sw
    nw = min(512, sw)       # matmul N width (f32r needs >=256)
    jn = sw // nw
    gxw = min(512, s)       # GX matmul chunk width
    sched = default_exp_sched(n_pass, tch, n_dve_exp)

    nc = bacc.Bacc("TRN2", target_bir_lowering=False, debug=False)

    xT = nc.dram_tensor("xT", [D, s], f32r, kind="ExternalInput").ap()
    gT = nc.dram_tensor("gT", [D, D], f32r, kind="ExternalInput").ap()
    wvT = nc.dram_tensor("wvT", [D, D + 2], f32r, kind="ExternalInput").ap()
    num_d = nc.dram_tensor("num", [D, sq], f32, kind="ExternalOutput").ap()
    den_d = nc.dram_tensor("den", [1, sq], f32, kind="ExternalOutput").ap()

    with tile.TileContext(nc) as tc, ExitStack() as ctx:
        const = ctx.enter_context(tc.tile_pool(name="const", bufs=1))
        big = ctx.enter_context(tc.tile_pool(name="big", bufs=1))
        exp_pool = ctx.enter_context(tc.tile_pool(name="exp", bufs=4))
        sch_pool = ctx.enter_context(tc.tile_pool(name="sch", bufs=3))
        vres_pool = ctx.enter_context(tc.tile_pool(name="vres", bufs=2))
        stage = ctx.enter_context(tc.tile_pool(name="stage", bufs=2))

        gT_sb = const.tile([D, D], f32r, tag="gT")
        wv_sb = const.tile([D, D + 2], f32r, tag="wv")
        ones8 = const.tile([128, 256], fp8, tag="ones8")
        alpha_sb = const.tile([128, tch], f32, tag="alpha")    # alpha - C
        alpha16 = const.tile([128, tch], f32, tag="alpha16")   # schraudolph bias

        xT_sb = big.tile([D, s], f32r, tag="xT")
        gx_sb = big.tile([D, s], f32r, tag="gx")
        v8hi = big.tile([128, s], fp8, tag="v8hi")   # [(pair g, two, e128)]
        v8lo = big.tile([128, s], fp8, tag="v8lo")

        # ---- input DMAs: few large transfers (SWDGE descriptor generation
        # is ~1us per dma_start and serializes on the queue engine)
        nc.sync.dma_start(gT_sb[:], gT)
        nc.sync.dma_start(wv_sb[:], wvT)
        xw = max(1024, s // 4)
        for st in range(0, s, xw):
            w = min(xw, s - st)
            nc.sync.dma_start(xT_sb[:, st:st + w], xT[:, st:st + w])
        nc.vector.memset(ones8[:], 1.0)

        # ---- phase A: GX projection + V/alpha, PSUM pools closed after
        qkv_ctx = ExitStack()
        gxps = qkv_ctx.enter_context(tc.tile_pool(name="gxps", bufs=3,
                                                  space="PSUM"))
        vps = qkv_ctx.enter_context(tc.tile_pool(name="vps", bufs=3,
                                                 space="PSUM"))

        def emit_gx(j):
            st, w = j * gxw, min(gxw, s - j * gxw)
            gp = gxps.tile([128, gxw], f32, tag="gx")
            nc.tensor.matmul(gp[:, :w], gT_sb[:], xT_sb[:, st:st + w])
            # ACT is idle before the first exp; keep DVE free for V-prep
            nc.scalar.copy(gx_sb[:, st:st + w], gp[:, :w])

        def emit_v(c):
            vp = vps.tile([128, D + 2], f32, tag="vp")
            xc = xT_sb[:, c * 128:(c + 1) * 128]
            nc.tensor.matmul(vp[:], xc, wv_sb[:])
            # GPSIMD cannot touch PSUM (walrus birverifier) -> all on DVE
            nc.vector.tensor_copy(v8hi[:, c * 128:(c + 1) * 128], vp[:, :D])
            vr = vres_pool.tile([128, D], f32, tag="vr")
            nc.vector.tensor_sub(vr[:], vp[:, :D],
                                 v8hi[:, c * 128:(c + 1) * 128])
            nc.vector.tensor_copy(v8lo[:, c * 128:(c + 1) * 128], vr[:])
            nc.vector.tensor_scalar_add(alpha_sb[:, c:c + 1],
                                        vp[:, D:D + 1], -CSHIFT)

        ngx = (s + gxw - 1) // gxw
        # GX chunk 0 + V chunks 0/1 feed the first scores/AV pair
        emit_gx(0)
        emit_v(0)
        emit_v(1)
        for j in range(1, ngx):
            emit_gx(j)
        for c in range(2, tch):
            emit_v(c)
        qkv_ctx.close()

        # schraudolph per-partition bias from alpha (single DVE op)
        nc.vector.tensor_scalar(alpha16[:], alpha_sb[:], SCH_A,
                                16256.5 + SCH_DELTA, ALU.mult, ALU.add)

        acc_ctx = ExitStack()
        scps = acc_ctx.enter_context(tc.tile_pool(name="scps", bufs=2,
                                                  space="PSUM"))
        accps = acc_ctx.enter_context(tc.tile_pool(name="accps", bufs=1,
                                                   space="PSUM"))
        denps = acc_ctx.enter_context(tc.tile_pool(name="denps", bufs=1,
                                                   space="PSUM"))

        ones3 = ones8[:, :].rearrange("p (two e) -> p two e", two=2)

        def emit_scores(p, c):
            sc = scps.tile([128, sw], f32, tag="sc")
            gxc = gx_sb[:, c * 128:(c + 1) * 128]
            for j in range(jn):
                nc.tensor.matmul(sc[:, j * nw:(j + 1) * nw], gxc,
                                 xT_sb[:, p * sw + j * nw:
                                       p * sw + (j + 1) * nw])
            return sc

        def emit_exp(p, c, sc, pair):
            """exp(SCALE*sc + alpha[c] - C) -> fp8 into pair slot c%2."""
            half = pair[:, (c % 2) * sw:(c % 2) * sw + sw]
            if sched[(p, c)] == "act":
                nc.scalar.activation(half, sc[:], FT.Exp,
                                     bias=alpha_sb[:, c:c + 1], scale=SCALE)
            else:
                # DVE schraudolph -> bf16 bits; bf16->fp8 on the DMA engines
                # (casting DMA, initiated from gpsimd; Pool compute is too
                # slow for casts and cannot touch PSUM)
                t16 = sch_pool.tile([128, sw], i16, tag="t16")
                nc.vector.tensor_scalar(t16[:], sc[:], SCALE * SCH_A,
                                        alpha16[:, c:c + 1],
                                        ALU.mult, ALU.add)
                nc.gpsimd.dma_start(half, t16[:].bitcast(bf16))

        # ---- attention passes
        pairs = {}
        for p in range(n_pass):
            acc_o = accps.tile([128, sw], f32, tag="acco")
            den_ps = denps.tile([128, sw], f32, tag="den")

            def emit_av(p, g, first, last):
                # same-stationary matmuls kept adjacent (ldweights reuse)
                pair3 = pairs.pop((p, g))[:, :].rearrange(
                    "p (two n) -> p two n", two=2)
                vh = v8hi[:, g * 256:(g + 1) * 256].rearrange(
                    "p (two e) -> p two e", two=2)
                vl = v8lo[:, g * 256:(g + 1) * 256].rearrange(
                    "p (two e) -> p two e", two=2)
                for st, kw in ((vh, dict(start=first, stop=False)),
                               (vl, dict(start=False, stop=last))):
                    for j in range(jn):
                        nc.tensor.matmul(acc_o[:, j * nw:(j + 1) * nw], st,
                                         pair3[:, :, j * nw:(j + 1) * nw],
                                         perf_mode=DR, **kw)
                for j in range(jn):
                    nc.tensor.matmul(den_ps[:, j * nw:(j + 1) * nw],
                                     ones3, pair3[:, :, j * nw:(j + 1) * nw],
                                     perf_mode=DR, start=first, stop=last)

            for c in range(tch):
                sc = emit_scores(p, c)
                g = c // 2
                if c % 2 == 0:
                    pairs[(p, g)] = exp_pool.tile([128, 2 * sw], fp8,
                                                  name="pair", tag="pair")
                emit_exp(p, c, sc, pairs[(p, g)])
                if c % 2 == 1:
                    emit_av(p, g, first=(g == 0), last=(g == npair - 1))

            # stage num/den to SBUF (Pool/DVE), DMA out
            num_sb = stage.tile([128, sw], f32, tag="num")
            den_sb = stage.tile([1, sw], f32, tag="densb")
            nc.vector.tensor_copy(num_sb[:], acc_o[:])
            nc.vector.tensor_copy(den_sb[:], den_ps[0:1, :])
            nc.sync.dma_start(num_d[:, p * sw:(p + 1) * sw], num_sb[:])
            nc.sync.dma_start(den_d[:, p * sw:(p + 1) * sw], den_sb[:])
        acc_ctx.close()
    nc.compile()
    return nc


def make_in_maps(x, Wq, bq, Wk, Wv, s=S, sq=SQ, n_cores=N_CORES):
    """Per-core inputs. Core c -> batch c//per_b, query half c%per_b via
    column rotation of x^T."""
    x = np.asarray(x, np.float64)
    nb = x.shape[0]
    per_b = n_cores // nb
    Wq = np.asarray(Wq, np.float64)
    Wk = np.asarray(Wk, np.float64)
    Wv = np.asarray(Wv, np.float64)
    bq = np.asarray(bq, np.float64)
    gT = (Wk.T @ Wq).astype(np.float32)               # [d', d]
    wtl = (SCALE * (Wk.T @ bq)).reshape(D, 1)
    wv_aug = np.concatenate([Wv.T, wtl, wtl], axis=1).astype(np.float32)
    maps = []
    for c in range(n_cores):
        b, h = c // per_b, c % per_b
        xt = np.ascontiguousarray(x[b].T.astype(np.float32))
        if h:
            xt = np.ascontiguousarray(
                np.concatenate([xt[:, h * sq:], xt[:, :h * sq]], axis=1))
        maps.append({"xT": xt, "gT": gT, "wvT": wv_aug})
    return maps


_NC_CACHE = {}


def _get_nc():
    if "nc" not in _NC_CACHE:
        _NC_CACHE["nc"] = build_attention_bass()
    return _NC_CACHE["nc"]


def postprocess(results, bv, x_shape=(B, S, D), n_cores=N_CORES, sq=SQ):
    """results[c] = {num: [D, sq], den: [1, sq]} -> full [B, S*D] output."""
    nb = x_shape[0]
    per_b = n_cores // nb
    bv = np.asarray(bv, np.float64).reshape(1, D)
    out = np.empty((nb, x_shape[1] * D), np.float32)
    for c in range(n_cores):
        b, h = c // per_b, c % per_b
        num = np.asarray(results[c]["num"], np.float64)   # [D, sq]
        den = np.asarray(results[c]["den"], np.float64)   # [1, sq]
        o = (num / den).T + bv                            # [sq, D]
        out[b, h * sq * D:(h + 1) * sq * D] = o.astype(np.float32).reshape(-1)
    return out


def run_on_hw(inputs, trace=False, **kw):
    from concourse.bass_utils import run_bass_kernel_spmd
    nc = _get_nc()
    maps = make_in_maps(inputs["x"], inputs["Wq"], inputs["bq"],
                        inputs["Wk"], inputs["Wv"])
    res = run_bass_kernel_spmd(nc, maps, core_ids=list(range(N_CORES)),
                               trace=trace, **kw)
    out = postprocess(res.results, inputs["bv"],
                      x_shape=np.asarray(inputs["x"]).shape)
    return out, res


def kernel(**inputs):
    out, _ = run_on_hw(inputs, trace=False)
    return out


# revision 23
# speedup vs baseline: 1.4902x; 1.1177x over previous
"""Trainium2 Bass kernel for nn_AttentionLayer (B=4, S=4096, D=128, fp32).

Sharding: batch (4) x query-half (2) across 8 NeuronCores; the query half is
realized by a host-side column ROTATION of x^T (keys are permutation
invariant under softmax+sum), so every core runs the identical SPMD program
with its queries at columns 0..sq-1.

Math restructure vs a direct port:
  scores[t,s] = q_s . k_t = x_s^T (Wq^T Wk) x_t
    -> precompute (host, fp64) gT = Wk^T Wq; on device GX = gT^T @ x^T once
       (a [128,4096] tensor), then scores chunks = GX-chunk^T @ x^T.
       This removes the Q and K projections, their PSUM->SBUF copies, and
       the duplicated query DMA of the v1 kernel.
  bq is folded into the exp bias alpha[t] = SCALE*bq.k_t (extra column of
    the V projection, as v1); bk cancels in softmax; bv is applied on host.
  exp is computed straight into fp8e4 (range centered by a global shift C,
    which cancels exactly in softmax); the AV and denominator matmuls then
    run in fp8 DoubleRow perf mode (2 contraction rows/partition, 0.5
    cycles/col). V is split into an fp8 hi+lo pair (error feedback), so V
    quantization contributes ~nothing; only the exp fp8 rounding remains.
  Normalization (num/den + bv) happens on host in fp64 for free.

Engine budget per core: PE ~123k cycles (scores 64k, AV+den 49k, proj 8k);
ACT all-exp would be ~80k cycles, so a slice of exp chunks is offloaded as
int16-Schraudolph (DVE/Pool tensor_scalar -> bf16 bit pattern) + bf16->fp8
convert on the other of the two engines.
"""

import sys

import numpy as np

for _p in ("/opt/trn_rl_repo", "/opt/pypackages"):
    if _p not in sys.path:
        sys.path.append(_p)

B, S, D = 4, 4096, 128
N_CORES = 8
SQ = S // 2            # queries per core
SCALE = 1.0 / float(np.sqrt(D))
CSHIFT = 1.5           # global exp shift: exp(y-C); cancels in softmax
# Schraudolph (bf16 bit pattern): i16 = y*184.6635 + 16256.5 + delta
SCH_A = 128.0 / float(np.log(2.0))
SCH_DELTA = -5.5       # centers the 2^frac linear-interp overestimate
LDW = True


def default_exp_sched(n_pass, tch, n_dve=0):
    """Per (pass, chunk) exp engine: 'act' or 'dve' (Schraudolph + DMA-cast).
    n_dve: int (same per pass) or per-pass list. Pass 0's DVE is busy with
    V-prep, so its offloaded chunks sit late in the pass."""
    if isinstance(n_dve, int):
        n_dve = [n_dve] * n_pass
    sched = {}
    npair = tch // 2
    for p in range(n_pass):
        nd = min(n_dve[p] if p < len(n_dve) else n_dve[-1], npair)
        # offloaded chunks are the ODD chunk of a pair, so ACT and DVE
        # compute the two exps of a pair CONCURRENTLY (PE stays fed).
        # Pass 0: use the back pairs (V-prep occupies DVE early).
        if p == 0:
            gset = set(range(npair - 2, max(-1, npair - 2 - nd), -1))
        else:
            gset = set(range(1, min(npair, 1 + nd)))
        for c in range(tch):
            g = c // 2
            sched[(p, c)] = "dve" if (g in gset and c % 2 == 1) else "act"
    return sched


def build_attention_bass(s=S, sq=SQ, sw=1024, n_dve_exp=(6, 12)):
    """Single-core SPMD program. s: keys; sq: queries; sw: pass width."""
    import concourse.bass as bass
    import concourse.mybir as mybir
    import concourse.tile as tile
    from concourse import bacc
    from contextlib import ExitStack

    f32 = mybir.dt.float32
    f32r = mybir.dt.float32r
    fp8 = mybir.dt.float8e4
    bf16 = mybir.dt.bfloat16
    i16 = mybir.dt.int16
    FT = mybir.ActivationFunctionType
    DR = mybir.MatmulPerfMode.DoubleRow
    ALU = mybir.AluOpType

    tch = s // 128          # key chunks (128 keys each)
    npair = tch // 2        # 256-key pair groups
    n_pass = sq // sw
    nw = min(512, sw)       # matmul N width (f32r needs >=256)
    jn = sw // nw
    gxw = min(512, s)       # GX matmul chunk width
    sched = default_exp_sched(n_pass, tch, n_dve_exp)

    nc = bacc.Bacc("TRN2", target_bir_lowering=False, debug=False)

    xT = nc.dram_tensor("xT", [D, s], f32r, kind="ExternalInput").ap()
    gT = nc.dram_tensor("gT", [D, D], f32r, kind="ExternalInput").ap()
    wvT = nc.dram_tensor("wvT", [D, D + 2], f32r, kind="ExternalInput").ap()
    num_d = nc.dram_tensor("num", [D, sq], f32, kind="ExternalOutput").ap()
    den_d = nc.dram_tensor("den", [1, sq], f32, kind="ExternalOutput").ap()

    with tile.TileContext(nc) as tc, ExitStack() as ctx:
        const = ctx.enter_context(tc.tile_pool(name="const", bufs=1))
        big = ctx.enter_context(tc.tile_pool(name="big", bufs=1))
        exp_pool = ctx.enter_context(tc.tile_pool(name="exp", bufs=4))
        sch_pool = ctx.enter_context(tc.tile_pool(name="sch", bufs=3))
        vres_pool = ctx.enter_context(tc.tile_pool(name="vres", bufs=2))
        stage = ctx.enter_context(tc.tile_pool(name="stage", bufs=2))

        gT_sb = const.tile([D, D], f32r, tag="gT")
        wv_sb = const.tile([D, D + 2], f32r, tag="wv")
        ones8 = const.tile([128, 256], fp8, tag="ones8")
        alpha_sb = const.tile([128, tch], f32, tag="alpha")    # alpha - C
        alpha16 = const.tile([128, tch], f32, tag="alpha16")   # schraudolph bias

        xT_sb = big.tile([D, s], f32r, tag="xT")
        gx_sb = big.tile([D, s], f32r, tag="gx")
        v8hi = big.tile([128, s], fp8, tag="v8hi")   # [(pair g, two, e128)]
        v8lo = big.tile([128, s], fp8, tag="v8lo")

        # ---- input DMAs: descriptor generation (~0.7us each) serializes
        # per queue engine, so spread the xT chunks across four queues
        nc.sync.dma_start(gT_sb[:], gT)
        nc.sync.dma_start(wv_sb[:], wvT)
        xw = max(1024, s // 4)
        qengs = [nc.sync, nc.scalar, nc.gpsimd, nc.sync]
        for i, st in enumerate(range(0, s, xw)):
            w = min(xw, s - st)
            qengs[i % 4].dma_start(xT_sb[:, st:st + w], xT[:, st:st + w])
        nc.vector.memset(ones8[:], 1.0)

        # ---- phase A: GX projection + V/alpha, PSUM pools closed after
        qkv_ctx = ExitStack()
        gxps = qkv_ctx.enter_context(tc.tile_pool(name="gxps", bufs=3,
                                                  space="PSUM"))
        vps = qkv_ctx.enter_context(tc.tile_pool(name="vps", bufs=3,
                                                 space="PSUM"))

        def emit_gx(j):
            st, w = j * gxw, min(gxw, s - j * gxw)
            gp = gxps.tile([128, gxw], f32, tag="gx")
            nc.tensor.matmul(gp[:, :w], gT_sb[:], xT_sb[:, st:st + w])
            # ACT is idle before the first exp; keep DVE free for V-prep
            nc.scalar.copy(gx_sb[:, st:st + w], gp[:, :w])

        # V-prep in groups of up to 8 chunks: per chunk only a PSUM->SBUF
        # f32 stage copy + alpha extract on DVE; the fp8 hi/lo casts are
        # bulk DMA-casts (one per group) + one group-wide DVE subtract.
        vgrp = min(8, tch)
        vstage = big.tile([128, s], f32, tag="vstage")

        def emit_v_chunk(c):
            vp = vps.tile([128, D + 2], f32, tag="vp")
            xc = xT_sb[:, c * 128:(c + 1) * 128]
            nc.tensor.matmul(vp[:], xc, wv_sb[:])
            nc.vector.tensor_copy(vstage[:, c * 128:(c + 1) * 128],
                                  vp[:, :D])
            nc.vector.tensor_scalar_add(alpha_sb[:, c:c + 1],
                                        vp[:, D:D + 1], -CSHIFT)

        def emit_v_group(g0):
            lo_, hi_ = g0 * 128, (g0 + vgrp) * 128
            st = vstage[:, lo_:hi_]
            vr = vres_pool.tile([128, vgrp * 128], f32, tag="vr")
            nc.gpsimd.dma_start(v8hi[:, lo_:hi_], st)          # f32->fp8
            nc.vector.tensor_sub(vr[:], st, v8hi[:, lo_:hi_])
            nc.gpsimd.dma_start(v8lo[:, lo_:hi_], vr[:])       # f32->fp8

        ngx = (s + gxw - 1) // gxw
        emit_gx(0)
        for c in range(vgrp):
            emit_v_chunk(c)
        emit_v_group(0)
        for j in range(1, ngx):
            emit_gx(j)
        for g0 in range(vgrp, tch, vgrp):
            for c in range(g0, g0 + vgrp):
                emit_v_chunk(c)
            emit_v_group(g0)
        qkv_ctx.close()

        # schraudolph per-partition bias from alpha (single DVE op)
        nc.vector.tensor_scalar(alpha16[:], alpha_sb[:], SCH_A,
                                16256.5 + SCH_DELTA, ALU.mult, ALU.add)

        acc_ctx = ExitStack()
        scps = acc_ctx.enter_context(tc.tile_pool(name="scps", bufs=2,
                                                  space="PSUM"))
        accps = acc_ctx.enter_context(tc.tile_pool(name="accps", bufs=1,
                                                   space="PSUM"))
        denps = acc_ctx.enter_context(tc.tile_pool(name="denps", bufs=1,
                                                   space="PSUM"))

        ones3 = ones8[:, :].rearrange("p (two e) -> p two e", two=2)

        def emit_scores(p, c):
            sc = scps.tile([128, sw], f32, tag="sc")
            gxc = gx_sb[:, c * 128:(c + 1) * 128]
            for j in range(jn):
                nc.tensor.matmul(sc[:, j * nw:(j + 1) * nw], gxc,
                                 xT_sb[:, p * sw + j * nw:
                                       p * sw + (j + 1) * nw])
            return sc

        def emit_exp(p, c, sc, pair):
            """exp(SCALE*sc + alpha[c] - C) -> fp8 into pair slot c%2."""
            half = pair[:, (c % 2) * sw:(c % 2) * sw + sw]
            if sched[(p, c)] == "act":
                nc.scalar.activation(half, sc[:], FT.Exp,
                                     bias=alpha_sb[:, c:c + 1], scale=SCALE)
            else:
                # DVE schraudolph -> bf16 bits; bf16->fp8 on the DMA engines
                # (casting DMA, initiated from gpsimd; Pool compute is too
                # slow for casts and cannot touch PSUM)
                t16 = sch_pool.tile([128, sw], i16, tag="t16")
                nc.vector.tensor_scalar(t16[:], sc[:], SCALE * SCH_A,
                                        alpha16[:, c:c + 1],
                                        ALU.mult, ALU.add)
                nc.gpsimd.dma_start(half, t16[:].bitcast(bf16))

        # ---- attention passes
        pairs = {}
        for p in range(n_pass):
            acc_o = accps.tile([128, sw], f32, tag="acco")
            den_ps = denps.tile([128, sw], f32, tag="den")

            def emit_av(p, g, first, last):
                # same-stationary matmuls kept adjacent (ldweights reuse)
                pair3 = pairs.pop((p, g))[:, :].rearrange(
                    "p (two n) -> p two n", two=2)
                vh = v8hi[:, g * 256:(g + 1) * 256].rearrange(
                    "p (two e) -> p two e", two=2)
                vl = v8lo[:, g * 256:(g + 1) * 256].rearrange(
                    "p (two e) -> p two e", two=2)
                for st, kw in ((vh, dict(start=first, stop=False)),
                               (vl, dict(start=False, stop=last))):
                    for j in range(jn):
                        nc.tensor.matmul(acc_o[:, j * nw:(j + 1) * nw], st,
                                         pair3[:, :, j * nw:(j + 1) * nw],
                                         perf_mode=DR, **kw)
                for j in range(jn):
                    nc.tensor.matmul(den_ps[:, j * nw:(j + 1) * nw],
                                     ones3, pair3[:, :, j * nw:(j + 1) * nw],
                                     perf_mode=DR, start=first, stop=last)

            # AV(g) is emitted one pair late so the PE never waits on the
            # exp/offload chain latency (it has pair g+1's scores to run)
            for c in range(tch):
                sc = emit_scores(p, c)
                g = c // 2
                if c % 2 == 0:
                    pairs[(p, g)] = exp_pool.tile([128, 2 * sw], fp8,
                                                  name="pair", tag="pair")
                emit_exp(p, c, sc, pairs[(p, g)])
                if c % 2 == 1 and g >= 1:
                    emit_av(p, g - 1, first=(g - 1 == 0), last=False)
            emit_av(p, npair - 1, first=(npair == 1), last=True)

            # stage num/den to SBUF (Pool/DVE), DMA out
            num_sb = stage.tile([128, sw], f32, tag="num")
            den_sb = stage.tile([1, sw], f32, tag="densb")
            nc.vector.tensor_copy(num_sb[:], acc_o[:])
            nc.vector.tensor_copy(den_sb[:], den_ps[0:1, :])
            nc.sync.dma_start(num_d[:, p * sw:(p + 1) * sw], num_sb[:])
            nc.sync.dma_start(den_d[:, p * sw:(p + 1) * sw], den_sb[:])
        acc_ctx.close()
    nc.compile()
    return nc


def make_in_maps(x, Wq, bq, Wk, Wv, s=S, sq=SQ, n_cores=N_CORES):
    """Per-core inputs. Core c -> batch c//per_b, query half c%per_b via
    column rotation of x^T."""
    x = np.asarray(x, np.float64)
    nb = x.shape[0]
    per_b = n_cores // nb
    Wq = np.asarray(Wq, np.float64)
    Wk = np.asarray(Wk, np.float64)
    Wv = np.asarray(Wv, np.float64)
    bq = np.asarray(bq, np.float64)
    gT = (Wk.T @ Wq).astype(np.float32)               # [d', d]
    wtl = (SCALE * (Wk.T @ bq)).reshape(D, 1)
    wv_aug = np.concatenate([Wv.T, wtl, wtl], axis=1).astype(np.float32)
    maps = []
    for c in range(n_cores):
        b, h = c // per_b, c % per_b
        xt = np.ascontiguousarray(x[b].T.astype(np.float32))
        if h:
            xt = np.ascontiguousarray(
                np.concatenate([xt[:, h * sq:], xt[:, :h * sq]], axis=1))
        maps.append({"xT": xt, "gT": gT, "wvT": wv_aug})
    return maps


_NC_CACHE = {}


def _get_nc():
    if "nc" not in _NC_CACHE:
        _NC_CACHE["nc"] = build_attention_bass()
    return _NC_CACHE["nc"]


def postprocess(results, bv, x_shape=(B, S, D), n_cores=N_CORES, sq=SQ):
    """results[c] = {num: [D, sq], den: [1, sq]} -> full [B, S*D] output."""
    nb = x_shape[0]
    per_b = n_cores // nb
    bv = np.asarray(bv, np.float64).reshape(1, D)
    out = np.empty((nb, x_shape[1] * D), np.float32)
    for c in range(n_cores):
        b, h = c // per_b, c % per_b
        num = np.asarray(results[c]["num"], np.float64)   # [D, sq]
        den = np.asarray(results[c]["den"], np.float64)   # [1, sq]
        o = (num / den).T + bv                            # [sq, D]
        out[b, h * sq * D:(h + 1) * sq * D] = o.astype(np.float32).reshape(-1)
    return out


def run_on_hw(inputs, trace=False, **kw):
    from concourse.bass_utils import run_bass_kernel_spmd
    nc = _get_nc()
    maps = make_in_maps(inputs["x"], inputs["Wq"], inputs["bq"],
                        inputs["Wk"], inputs["Wv"])
    res = run_bass_kernel_spmd(nc, maps, core_ids=list(range(N_CORES)),
                               trace=trace, **kw)
    out = postprocess(res.results, inputs["bv"],
                      x_shape=np.asarray(inputs["x"]).shape)
    return out, res


def kernel(**inputs):
    out, _ = run_on_hw(inputs, trace=False)
    return out
